# revision 16
# baseline (speedup 1.0000x reference)
"""Mixtral sparse-MoE block on 8 Trainium2 NeuronCores.

Strategy: expert-parallel. The router (tiny: T*H*E = 34 MFLOP) runs on
host in float64; tokens are gathered per expert on host; each NeuronCore
runs one expert's FFN over its (padded) token set; host scatter-adds the
weighted expert outputs back.

Device kernel per core (cap = padded token count, H=2048, F=7168):
  mm1:  gT[2F, cap] = w1_e.T tiles @ xT        (float32r, full PE rate)
  h:    hT[F, cap]  = silu(gate) * up          (ScalarE silu + VectorE mul)
  mm2:  yT[H, cap]  = w2_e.T tiles @ hT        (fp16 operands, fp32 accum)

Layouts are transposed (feature-major) so both matmuls consume natural
[K=128, M=128] stationary tiles and produce the next stage's moving
operand directly - no on-device transposes.
"""

import numpy as np

P = 128
H = 2048
F = 7168
E = 8
TOPK = 2
NCORES = 8

# Filled by kernel() after each run; test harness reads exec_time_ns.
LAST_RESULTS = None


def _chunks_for(cap: int) -> list[tuple[int, int]]:
    """Split the token axis into matmul moving-dim chunks <=512 wide.

    fp32r matmuls hit full PE rate only for moving dim >=256, so split
    as evenly as possible (cap>=512 makes every chunk >=256) instead of
    512+remainder.
    """
    n = -(-cap // 512)
    out = []
    c0 = 0
    rem = cap
    for i in range(n, 0, -1):
        s = (rem + i - 1) // i
        s = min((s + 1) // 2 * 2, rem)  # even width: odd fails the ISA check
        out.append((c0, s))
        c0 += s
        rem -= s
    return out


def build_moe_ffn(h: int, f: int, cap: int, chunks: list[tuple[int, int]]):
    """One-expert FFN bass program, run SPMD on all 8 cores."""
    import concourse.tile as tile
    from concourse import bacc, mybir

    k1 = h // P   # contraction tiles of mm1 (hidden dim)
    j = f // P    # ffn tiles (= contraction tiles of mm2)
    no = h // P   # output tiles (hidden dim)

    f32r = mybir.dt.float32r
    f32 = mybir.dt.float32
    f16 = mybir.dt.float16

    # Bacc (not plain Bass): its compile() runs generate_event_semaphores,
    # which splits multi-sem waits — walrus allows 1 wait per instruction.
    nc = bacc.Bacc(None)
    xt_d = nc.dram_tensor("xt", [k1, P, cap], f32r, kind="ExternalInput")
    w1_d = nc.dram_tensor("w1t", [2 * j, P, k1 * P], f32r, kind="ExternalInput")
    w2_d = nc.dram_tensor("w2t", [no, P, j * P], f16, kind="ExternalInput")
    yt_d = nc.dram_tensor("yt", [no, P, cap], f32, kind="ExternalOutput")

    with tile.TileContext(nc) as tc:
        with (
            tc.tile_pool(name="xtp", bufs=1) as xt_pool,
            tc.tile_pool(name="htp", bufs=1) as ht_pool,
            tc.tile_pool(name="w1p", bufs=2) as w1_pool,
            tc.tile_pool(name="w2p", bufs=2) as w2_pool,
            tc.tile_pool(name="tmp", bufs=3) as tmp_pool,
            tc.tile_pool(name="yp", bufs=3) as y_pool,
            tc.tile_pool(name="ps", bufs=2, space="PSUM") as ps_pool,
        ):
            def load_w1_pair(jj, split=1):
                # split>1 loads in k-tile groups so the first matmul's dep
                # clears after 1/split of the transfer (startup latency).
                w1g = w1_pool.tile([P, k1 * P], f32r, tag="w1g")
                w1u = w1_pool.tile([P, k1 * P], f32r, tag="w1u")
                step = k1 * P // split
                for t, buf in ((w1g, w1_d[jj]), (w1u, w1_d[j + jj])):
                    for s in range(split):
                        nc.sync.dma_start(
                            t[:, s * step:(s + 1) * step],
                            buf[:, s * step:(s + 1) * step],
                        )
                return w1g, w1u

            # Emit j=0 weight loads BEFORE the xt loads so their DMAs land at
            # the queue heads - the first matmul's critical path is w1[j=0].
            w1_first = load_w1_pair(0, split=4)

            xt = xt_pool.tile([P, k1 * cap], f32r)
            for k in range(k1):
                nc.sync.dma_start(xt[:, k * cap:(k + 1) * cap], xt_d[k])

            ht = ht_pool.tile([P, j * cap], f16)

            for jj in range(j):
                w1g, w1u = w1_first if jj == 0 else load_w1_pair(jj)
                # chunk-innermost: consecutive matmuls share the stationary
                # [128,128] weight tile, halving distinct LDWEIGHTS streams.
                pgs_ = [
                    ps_pool.tile([P, cw], f32, tag=f"ps{ci}", name=f"pg{ci}")
                    for ci, (c0, cw) in enumerate(chunks)
                ]
                for k in range(k1):
                    for ci, (c0, cw) in enumerate(chunks):
                        nc.tensor.matmul(
                            pgs_[ci][:],
                            w1g[:, k * P:(k + 1) * P],
                            xt[:, k * cap + c0:k * cap + c0 + cw],
                            start=(k == 0),
                            stop=(k == k1 - 1),
                        )
                pus_ = [
                    ps_pool.tile([P, cw], f32, tag=f"ps{ci}", name=f"pu{ci}")
                    for ci, (c0, cw) in enumerate(chunks)
                ]
                for k in range(k1):
                    for ci, (c0, cw) in enumerate(chunks):
                        nc.tensor.matmul(
                            pus_[ci][:],
                            w1u[:, k * P:(k + 1) * P],
                            xt[:, k * cap + c0:k * cap + c0 + cw],
                            start=(k == 0),
                            stop=(k == k1 - 1),
                        )
                for ci, (c0, cw) in enumerate(chunks):
                    pg, pu = pgs_[ci], pus_[ci]
                    # silu(pg) * pu, structured so every DVE op carries at
                    # most ONE cross-engine wait (walrus TT struct limit).
                    sig = tmp_pool.tile([P, cw], f32, tag="sig", name="sig")
                    nc.scalar.activation(
                        sig[:], pg[:], mybir.ActivationFunctionType.Sigmoid
                    )
                    pgs = tmp_pool.tile([P, cw], f32, tag="pgs", name="pgs")
                    nc.vector.tensor_copy(pgs[:], pg[:])
                    pus = tmp_pool.tile([P, cw], f32, tag="pus", name="pus")
                    nc.vector.tensor_copy(pus[:], pu[:])
                    sil = tmp_pool.tile([P, cw], f32, tag="sil", name="sil")
                    nc.vector.tensor_tensor(
                        sil[:], pgs[:], sig[:], mybir.AluOpType.mult
                    )
                    nc.vector.tensor_tensor(
                        ht[:, jj * cap + c0:jj * cap + c0 + cw],
                        sil[:],
                        pus[:],
                        mybir.AluOpType.mult,
                    )

            for nn in range(no):
                w2n = w2_pool.tile([P, j * P], f16)
                nc.sync.dma_start(w2n[:], w2_d[nn])
                pos_ = [
                    ps_pool.tile([P, cw], f32, tag=f"ps{ci}", name=f"po{ci}")
                    for ci, (c0, cw) in enumerate(chunks)
                ]
                for k in range(j):
                    for ci, (c0, cw) in enumerate(chunks):
                        nc.tensor.matmul(
                            pos_[ci][:],
                            w2n[:, k * P:(k + 1) * P],
                            ht[:, k * cap + c0:k * cap + c0 + cw],
                            start=(k == 0),
                            stop=(k == j - 1),
                        )
                for ci, (c0, cw) in enumerate(chunks):
                    yo = y_pool.tile([P, cw], f32, tag="yo", name="yo")
                    nc.scalar.copy(yo[:], pos_[ci][:])
                    nc.sync.dma_start(yt_d[nn, :, c0:c0 + cw], yo[:])

    nc.finalize()
    return nc


def _route(x: np.ndarray, gate_w: np.ndarray):
    """Host router in float64: logits, softmax, top-2, renormalize."""
    logits64 = x.astype(np.float64) @ gate_w.astype(np.float64)
    z = logits64 - logits64.max(-1, keepdims=True)
    p = np.exp(z)
    p /= p.sum(-1, keepdims=True)
    sel = np.argsort(-p, axis=-1, kind="stable")[:, :TOPK]
    pw = np.take_along_axis(p, sel, axis=-1)
    pw = pw / pw.sum(-1, keepdims=True)
    return logits64.astype(np.float32), sel, pw


def _maybe_enable_ldw_opt():
    """Opt-in experiment: let walrus dedupe back-to-back LDWEIGHTS of the
    same stationary tile (our matmuls are chunk-innermost so pairs share
    weights). Gated on MOE_LDW_OPT=1."""
    import os

    if os.environ.get("MOE_LDW_OPT") != "1":
        return
    import concourse.bass_utils as bu

    if getattr(bu, "_moe_ldw_patched", False):
        return
    orig = bu.run_command

    def patched(argv, **kwargs):
        argv = [
            a.replace("--enable-ldw-opt=false", "--enable-ldw-opt=true")
            if isinstance(a, str) else a
            for a in argv
        ]
        return orig(argv, **kwargs)

    bu.run_command = patched
    bu._moe_ldw_patched = True


def kernel(**inputs) -> tuple[np.ndarray, np.ndarray]:
    global LAST_RESULTS
    _maybe_enable_ldw_opt()
    from concourse.bass_utils import run_bass_kernel_spmd

    hidden = np.ascontiguousarray(np.asarray(inputs["hidden_states"], dtype=np.float32))
    gate_w = np.asarray(inputs["gate_w"], dtype=np.float32)
    w1 = np.asarray(inputs["w1"], dtype=np.float32)
    w2 = np.asarray(inputs["w2"], dtype=np.float32)

    b, s, h = hidden.shape
    t = b * s
    f = w2.shape[1]
    x = hidden.reshape(t, h)

    router_logits, sel, pw = _route(x, gate_w)

    tok_idx = []
    tok_wts = []
    for e in range(E):
        mask = sel == e                       # [T, TOPK], <=1 True per row
        toks = np.nonzero(mask.any(axis=1))[0]
        tok_idx.append(toks)
        tok_wts.append(pw[mask].astype(np.float32))
    counts = [len(ti) for ti in tok_idx]

    cap = max(256, -(-max(counts) // 4) * 4)
    chunks = _chunks_for(cap)
    k1 = h // P
    j = f // P
    no = h // P

    in_maps = []
    for e in range(E):
        xe = np.zeros((cap, h), np.float32)
        xe[:counts[e]] = x[tok_idx[e]]
        xt = np.ascontiguousarray(xe.T).reshape(k1, P, cap)
        w1t = np.ascontiguousarray(
            w1[e].reshape(k1, P, 2 * j, P).transpose(2, 1, 0, 3)
        ).reshape(2 * j, P, k1 * P)
        w2t = (
            np.ascontiguousarray(w2[e].reshape(j, P, no, P).transpose(2, 1, 0, 3))
            .reshape(no, P, j * P)
            .astype(np.float16)
        )
        in_maps.append({"xt": xt, "w1t": w1t, "w2t": w2t})

    nc = build_moe_ffn(h, f, cap, chunks)
    res = run_bass_kernel_spmd(nc, in_maps, list(range(NCORES)))
    LAST_RESULTS = res

    out = np.zeros((t, h), np.float32)
    for e in range(E):
        if counts[e] == 0:
            continue
        ye = res.results[e]["yt"].reshape(h, cap)[:, :counts[e]]
        out[tok_idx[e]] += tok_wts[e][:, None] * ye.T

    return out.reshape(b, s, h), router_logits


# revision 17
# speedup vs baseline: 1.0205x; 1.0205x over previous
"""Mixtral sparse-MoE block on 8 Trainium2 NeuronCores.

Strategy: expert-parallel. The router (tiny: T*H*E = 34 MFLOP) runs on
host in float64; tokens are gathered per expert on host; each NeuronCore
runs one expert's FFN over its (padded) token set; host scatter-adds the
weighted expert outputs back.

Device kernel per core (cap = padded token count, H=2048, F=7168):
  mm1:  gT[2F, cap] = w1_e.T tiles @ xT        (float32r, full PE rate)
  h:    hT[F, cap]  = silu(gate) * up          (ScalarE silu + VectorE mul)
  mm2:  yT[H, cap]  = w2_e.T tiles @ hT        (fp16 operands, fp32 accum)

Layouts are transposed (feature-major) so both matmuls consume natural
[K=128, M=128] stationary tiles and produce the next stage's moving
operand directly - no on-device transposes.
"""

import numpy as np

P = 128
H = 2048
F = 7168
E = 8
TOPK = 2
NCORES = 8

# Filled by kernel() after each run; test harness reads exec_time_ns.
LAST_RESULTS = None


def _chunks_for(cap: int) -> list[tuple[int, int]]:
    """Split the token axis into matmul moving-dim chunks <=512 wide.

    fp32r matmuls hit full PE rate only for moving dim >=256, so split
    as evenly as possible (cap>=512 makes every chunk >=256) instead of
    512+remainder.
    """
    n = -(-cap // 512)
    out = []
    c0 = 0
    rem = cap
    for i in range(n, 0, -1):
        s = (rem + i - 1) // i
        s = min((s + 1) // 2 * 2, rem)  # even width: odd fails the ISA check
        out.append((c0, s))
        c0 += s
        rem -= s
    return out


def build_moe_ffn(h: int, f: int, cap: int, chunks: list[tuple[int, int]]):
    """One-expert FFN bass program, run SPMD on all 8 cores."""
    import concourse.tile as tile
    from concourse import bacc, mybir

    k1 = h // P   # contraction tiles of mm1 (hidden dim)
    j = f // P    # ffn tiles (= contraction tiles of mm2)
    no = h // P   # output tiles (hidden dim)

    f32r = mybir.dt.float32r
    f32 = mybir.dt.float32
    f16 = mybir.dt.float16

    # Bacc (not plain Bass): its compile() runs generate_event_semaphores,
    # which splits multi-sem waits — walrus allows 1 wait per instruction.
    nc = bacc.Bacc(None)
    xt_d = nc.dram_tensor("xt", [k1, P, cap], f32r, kind="ExternalInput")
    w1_d = nc.dram_tensor("w1t", [2 * j, P, k1 * P], f32r, kind="ExternalInput")
    w2_d = nc.dram_tensor("w2t", [no, P, j * P], f16, kind="ExternalInput")
    yt_d = nc.dram_tensor("yt", [no, P, cap], f32, kind="ExternalOutput")

    with tile.TileContext(nc) as tc:
        with (
            tc.tile_pool(name="xtp", bufs=1) as xt_pool,
            tc.tile_pool(name="htp", bufs=1) as ht_pool,
            tc.tile_pool(name="w1p", bufs=2) as w1_pool,
            tc.tile_pool(name="w2p", bufs=2) as w2_pool,
            tc.tile_pool(name="tmp", bufs=3) as tmp_pool,
            tc.tile_pool(name="yp", bufs=3) as y_pool,
            tc.tile_pool(name="ps", bufs=2, space="PSUM") as ps_pool,
        ):
            def load_w1_pair(jj):
                w1g = w1_pool.tile([P, k1 * P], f32r, tag="w1g")
                nc.sync.dma_start(w1g[:], w1_d[jj])
                w1u = w1_pool.tile([P, k1 * P], f32r, tag="w1u")
                nc.sync.dma_start(w1u[:], w1_d[j + jj])
                return w1g, w1u

            # Emit j=0 weight loads BEFORE the xt loads so their DMAs land at
            # the queue heads - the first matmul's critical path is w1[j=0].
            w1_first = load_w1_pair(0)

            xt = xt_pool.tile([P, k1 * cap], f32r)
            for k in range(k1):
                nc.sync.dma_start(xt[:, k * cap:(k + 1) * cap], xt_d[k])

            ht = ht_pool.tile([P, j * cap], f16)

            for jj in range(j):
                w1g, w1u = w1_first if jj == 0 else load_w1_pair(jj)
                for (c0, cw) in chunks:
                    pg = ps_pool.tile([P, cw], f32)
                    for k in range(k1):
                        nc.tensor.matmul(
                            pg[:],
                            w1g[:, k * P:(k + 1) * P],
                            xt[:, k * cap + c0:k * cap + c0 + cw],
                            start=(k == 0),
                            stop=(k == k1 - 1),
                        )
                    pu = ps_pool.tile([P, cw], f32)
                    for k in range(k1):
                        nc.tensor.matmul(
                            pu[:],
                            w1u[:, k * P:(k + 1) * P],
                            xt[:, k * cap + c0:k * cap + c0 + cw],
                            start=(k == 0),
                            stop=(k == k1 - 1),
                        )
                    # silu(pg) * pu, structured so every DVE op carries at
                    # most ONE cross-engine wait (walrus TT struct limit).
                    sig = tmp_pool.tile([P, cw], f32)
                    nc.scalar.activation(
                        sig[:], pg[:], mybir.ActivationFunctionType.Sigmoid
                    )
                    pgs = tmp_pool.tile([P, cw], f32)
                    nc.vector.tensor_copy(pgs[:], pg[:])
                    pus = tmp_pool.tile([P, cw], f32)
                    nc.vector.tensor_copy(pus[:], pu[:])
                    sil = tmp_pool.tile([P, cw], f32)
                    nc.vector.tensor_tensor(
                        sil[:], pgs[:], sig[:], mybir.AluOpType.mult
                    )
                    nc.vector.tensor_tensor(
                        ht[:, jj * cap + c0:jj * cap + c0 + cw],
                        sil[:],
                        pus[:],
                        mybir.AluOpType.mult,
                    )

            for nn in range(no):
                w2n = w2_pool.tile([P, j * P], f16)
                nc.sync.dma_start(w2n[:], w2_d[nn])
                for (c0, cw) in chunks:
                    po = ps_pool.tile([P, cw], f32)
                    for k in range(j):
                        nc.tensor.matmul(
                            po[:],
                            w2n[:, k * P:(k + 1) * P],
                            ht[:, k * cap + c0:k * cap + c0 + cw],
                            start=(k == 0),
                            stop=(k == j - 1),
                        )
                    yo = y_pool.tile([P, cw], f32)
                    nc.scalar.copy(yo[:], po[:])
                    nc.sync.dma_start(yt_d[nn, :, c0:c0 + cw], yo[:])

    nc.finalize()
    return nc


def _route(x: np.ndarray, gate_w: np.ndarray):
    """Host router in float64: logits, softmax, top-2, renormalize."""
    logits64 = x.astype(np.float64) @ gate_w.astype(np.float64)
    z = logits64 - logits64.max(-1, keepdims=True)
    p = np.exp(z)
    p /= p.sum(-1, keepdims=True)
    sel = np.argsort(-p, axis=-1, kind="stable")[:, :TOPK]
    pw = np.take_along_axis(p, sel, axis=-1)
    pw = pw / pw.sum(-1, keepdims=True)
    return logits64.astype(np.float32), sel, pw


def kernel(**inputs) -> tuple[np.ndarray, np.ndarray]:
    global LAST_RESULTS
    from concourse.bass_utils import run_bass_kernel_spmd

    hidden = np.ascontiguousarray(np.asarray(inputs["hidden_states"], dtype=np.float32))
    gate_w = np.asarray(inputs["gate_w"], dtype=np.float32)
    w1 = np.asarray(inputs["w1"], dtype=np.float32)
    w2 = np.asarray(inputs["w2"], dtype=np.float32)

    b, s, h = hidden.shape
    t = b * s
    f = w2.shape[1]
    x = hidden.reshape(t, h)

    router_logits, sel, pw = _route(x, gate_w)

    tok_idx = []
    tok_wts = []
    for e in range(E):
        mask = sel == e                       # [T, TOPK], <=1 True per row
        toks = np.nonzero(mask.any(axis=1))[0]
        tok_idx.append(toks)
        tok_wts.append(pw[mask].astype(np.float32))
    counts = [len(ti) for ti in tok_idx]

    cap = max(256, -(-max(counts) // 4) * 4)
    chunks = _chunks_for(cap)
    k1 = h // P
    j = f // P
    no = h // P

    in_maps = []
    for e in range(E):
        xe = np.zeros((cap, h), np.float32)
        xe[:counts[e]] = x[tok_idx[e]]
        xt = np.ascontiguousarray(xe.T).reshape(k1, P, cap)
        w1t = np.ascontiguousarray(
            w1[e].reshape(k1, P, 2 * j, P).transpose(2, 1, 0, 3)
        ).reshape(2 * j, P, k1 * P)
        w2t = (
            np.ascontiguousarray(w2[e].reshape(j, P, no, P).transpose(2, 1, 0, 3))
            .reshape(no, P, j * P)
            .astype(np.float16)
        )
        in_maps.append({"xt": xt, "w1t": w1t, "w2t": w2t})

    nc = build_moe_ffn(h, f, cap, chunks)
    res = run_bass_kernel_spmd(nc, in_maps, list(range(NCORES)))
    LAST_RESULTS = res

    out = np.zeros((t, h), np.float32)
    for e in range(E):
        if counts[e] == 0:
            continue
        ye = res.results[e]["yt"].reshape(h, cap)[:, :counts[e]]
        out[tok_idx[e]] += tok_wts[e][:, None] * ye.T

    return out.reshape(b, s, h), router_logits
